# revision 43
# baseline (speedup 1.0000x reference)
"""Decoder layer (RMSNorm + RoPE causal attention + SwiGLU MLP) on 8 TRN2
NeuronCores.

Attention is tensor-parallel over heads (2 heads/core); scores are computed
in [k, q] layout (stationary K feature-tile, moving Q chunk) so no
probability transposes are needed. Wo partials are produced token-major and
ReduceScattered so each core ends up owning 4 x 128 tokens; the MLP then
runs fully locally per core on those 512 tokens with full-size (replicated,
streamed) gate/up/down weights. No AllReduce, no full-activation bounce.

kernel(**inputs) takes the full unsharded inputs and returns the full output.
"""

import math
import numpy as np
from contextlib import ExitStack

import concourse.bass as bass
import concourse.mybir as mybir
import concourse.tile as tile
from concourse import bacc, bass_utils
from concourse.masks import make_identity

f32 = mybir.dt.float32
f16 = mybir.dt.float16

NCORES = 8
P = 128
TCH = 512          # matmul moving free-dim chunk (tokens)
BASE = 10000.0
EPS = 1e-6
EXP_BIAS = -4.0    # constant bias for exp (replaces per-row max subtraction)

FULL_CFG = dict(B=2, T=2048, D=2048, H=16, FF=8192)


def _derive(cfg):
    B, T, D, H, FF = cfg["B"], cfg["T"], cfg["D"], cfg["H"], cfg["FF"]
    assert B == 2
    d = dict(cfg)
    d["HD"] = D // H
    assert d["HD"] == P
    d["N"] = B * T            # total tokens
    d["NH"] = H // NCORES     # heads per core
    d["DH"] = d["NH"] * P     # q/k/v width per core
    d["KD"] = D // P          # contraction chunks over D
    d["FFT"] = FF // P        # ff tiles (full, replicated MLP)
    d["CC"] = T // TCH        # token chunks per batch element
    d["QT"] = T // P          # 128-token tiles per batch element
    d["NTC"] = d["N"] // TCH  # token chunks total
    d["NAR"] = 4              # reduce-scatter groups
    d["GPA"] = d["NTC"] // d["NAR"]   # token chunks per RS group
    d["GT"] = d["N"] // d["NAR"]      # tokens per RS group
    d["FC"] = D // TCH        # feature chunks of the model dim
    assert d["GT"] // NCORES == P     # own tokens per group == P
    return d


def build_decoder(cfg):
    """Emit the bass program for one core (SPMD across 8)."""
    c = _derive(cfg)
    B, T, D, N = c["B"], c["T"], c["D"], c["N"]
    NH, DH = c["NH"], c["DH"]
    KD, CC, QT, FFT = c["KD"], c["CC"], c["QT"], c["FFT"]
    NAR, GPA, GT, FC = c["NAR"], c["GPA"], c["GT"], c["FC"]
    NM = 3 * NH               # q/k/v output tiles per core
    rgroups = [list(range(NCORES))]

    nc = bacc.Bacc("TRN2", target_bir_lowering=False, debug=False,
                   num_devices=NCORES)

    # ---- I/O ----
    xT16 = nc.dram_tensor("xT16", [D, N], f16, kind="ExternalInput")
    xown = nc.dram_tensor("xown", [NAR * P, D], f16, kind="ExternalInput")
    cq = nc.dram_tensor("cq", [P, N], f16, kind="ExternalInput")
    sq = nc.dram_tensor("sq", [P, N], f16, kind="ExternalInput")
    ck = nc.dram_tensor("ck", [P, N], f16, kind="ExternalInput")
    sk = nc.dram_tensor("sk", [P, N], f16, kind="ExternalInput")
    maskd = nc.dram_tensor("maskd", [P, P], f32, kind="ExternalInput")
    rotmd = nc.dram_tensor("rotmd", [P, P], f16, kind="ExternalInput")
    wqkv = nc.dram_tensor("wqkv", [3 * DH, KD * P], f16, kind="ExternalInput")
    wo = nc.dram_tensor("wo", [DH, D], f16, kind="ExternalInput")
    wgd = nc.dram_tensor("wgd", [FFT * P, KD * P], f16, kind="ExternalInput")
    wud = nc.dram_tensor("wud", [FFT * P, KD * P], f16, kind="ExternalInput")
    wdd = nc.dram_tensor("wdd", [cfg["FF"], D], f16, kind="ExternalInput")
    yOut = nc.dram_tensor("yOut", [NAR * P, D], f32, kind="ExternalOutput")

    # collective buffers (token-major Wo partials per RS group)
    p1g = [nc.dram_tensor(f"p1g_{g}", [GT, D], f16) for g in range(NAR)]
    a1g = [nc.dram_tensor(f"a1g_{g}", [P, D], f16) for g in range(NAR)]

    with tile.TileContext(nc, pool_alloc_mode="queue") as tc, ExitStack() as ctx:
        constp = ctx.enter_context(tc.tile_pool(name="constp", bufs=1))

        ones_pp = constp.tile([P, P], f16)
        nc.vector.memset(ones_pp, 1.0)
        mask_sb = constp.tile([P, P], f32)
        nc.sync.dma_start(mask_sb, maskd[:, :])
        epsP = constp.tile([P, 1], f32)
        nc.vector.memset(epsP, EPS)
        ebias = constp.tile([P, 1], f32)
        nc.vector.memset(ebias, EXP_BIAS)
        wo_sb = constp.tile([P, NH, D], f16)
        nc.sync.dma_start(wo_sb, wo.ap().rearrange("(h p) m -> p h m", p=P))

        # long-lived MLP tiles live on the right side of the SBUF ring so the
        # phase pools (left side) can alloc/release independently
        mkeep = tc.alloc_tile_pool(name="mkeep", bufs=1, side="right")
        x1g = [mkeep.tile([P, D], f16, name=f"x1g{g}", tag=f"x1g{g}")
               for g in range(NAR)]
        x1nT = [mkeep.tile([P, NAR * P], f16, name=f"xnt{i}", tag=f"xnt{i}")
                for i in range(KD)]

        persist = tc.alloc_tile_pool(name="persist", bufs=1)
        # rope'd q,k feature-major per head; v token-major per head
        qk_f = [persist.tile([P, N], f16, name=f"qkf{m}", tag=f"qkf{m}")
                for m in range(2 * NH)]
        v_sb = [persist.tile([P, N], f16, name=f"vsb{h}", tag=f"vsb{h}")
                for h in range(NH)]

        # single PSUM pool for the QKV + attention phases (8 banks):
        #   acc(3): qkv projection pair + attention AV accumulator
        #   sc(2):  score tiles
        #   rw(3):  ones-broadcast rowsums (rmsnorm + softmax) + Wo partials
        psx = tc.alloc_tile_pool(name="psx", bufs=1, space="PSUM")
        ap_ = tc.alloc_tile_pool(name="attntrans", bufs=1)
        qp = tc.alloc_tile_pool(name="qkvtrans", bufs=1)
        QGRP = TCH // P  # 128-token k-tiles per query chunk

        # ================= QKV (+ first RMSNorm) =================
        def qkv_setup(half):
            """Stream x^T for this half and compute 1/rms per token chunk."""
            toff = half * T
            x_sb = []
            for i in range(KD):
                xt = qp.tile([P, T], f16, name=f"xh{i}", tag="xh", bufs=KD)
                nc.sync.dma_start(xt, xT16[i * P:(i + 1) * P, toff:toff + T])
                x_sb.append(xt)
            rsb1 = []
            for cc in range(CC):
                rbq = psx.tile([P, TCH], f32, name="rbq", tag="rw", bufs=3)
                for i in range(KD):
                    x2 = qp.tile([P, TCH], f16, name="x2", tag="x2", bufs=1)
                    nc.vector.tensor_mul(x2, x_sb[i][:, cc * TCH:(cc + 1) * TCH],
                                         x_sb[i][:, cc * TCH:(cc + 1) * TCH])
                    nc.tensor.matmul(rbq, ones_pp, x2,
                                     start=(i == 0), stop=(i == KD - 1))
                srt = qp.tile([P, TCH], f32, name="srt", tag="srt", bufs=1)
                nc.scalar.activation(srt, rbq,
                                     mybir.ActivationFunctionType.Sqrt,
                                     bias=epsP[:, :], scale=1.0 / D)
                rsb = qp.tile([P, TCH], f16, name="rsb", tag="rsb", bufs=CC)
                with nc.allow_low_precision(reason="rmsnorm 1/rms"):
                    nc.vector.reciprocal(rsb, srt)
                rsb1.append(rsb)
            return x_sb, rsb1

        def qkv_mtile(half, m, x_sb, rsb1):
            """One q/k/v output tile (128 wide) over this half's tokens."""
            toff = half * T
            wt = qp.tile([P, KD * P], f16, name="wt", tag="wt", bufs=2)
            nc.sync.dma_start(wt, wqkv[m * P:(m + 1) * P, :])
            for ccp in range(0, CC, 2):
                npair = min(2, CC - ccp)
                pss = [psx.tile([P, TCH], f32, name="qkp", tag="acc", bufs=3)
                       for _ in range(npair)]
                for i in range(KD):
                    for u in range(npair):
                        cc = ccp + u
                        nc.tensor.matmul(
                            pss[u], wt[:, i * P:(i + 1) * P],
                            x_sb[i][:, cc * TCH:(cc + 1) * TCH],
                            start=(i == 0), stop=(i == KD - 1))
                for u in range(npair):
                    cc = ccp + u
                    gsl = slice(toff + cc * TCH, toff + (cc + 1) * TCH)
                    if m < 2 * NH:
                        # q or k head: scale by rs, apply rope; rotate-half is
                        # a +-64-partition shift done with two SBUF-SBUF DMAs
                        # (the sign lives in the host-prepared sin tables)
                        isq = m < NH
                        cd, sd = (cq, sq) if isq else (ck, sk)
                        ct = qp.tile([P, TCH], f16, name="ct", tag="ct",
                                     bufs=2)
                        nc.sync.dma_start(ct, cd[:, gsl])
                        st = qp.tile([P, TCH], f16, name="st", tag="st",
                                     bufs=2)
                        nc.sync.dma_start(st, sd[:, gsl])
                        qh = qp.tile([P, TCH], f16, name="qh", tag="qh",
                                     bufs=2)
                        nc.vector.tensor_tensor(qh, pss[u], rsb1[cc],
                                                mybir.AluOpType.mult)
                        qr = qp.tile([P, TCH], f16, name="qr", tag="qr",
                                     bufs=2)
                        hp = P // 2
                        nc.sync.dma_start(qr[0:hp, :], qh[hp:P, :])
                        nc.sync.dma_start(qr[hp:P, :], qh[0:hp, :])
                        t1 = qp.tile([P, TCH], f16, name="t1", tag="t1",
                                     bufs=2)
                        nc.vector.tensor_mul(t1, qh, ct)
                        t2 = qp.tile([P, TCH], f16, name="t2", tag="t2",
                                     bufs=2)
                        nc.vector.tensor_mul(t2, qr, st)
                        nc.vector.tensor_add(qk_f[m][:, gsl], t1, t2)
                    else:
                        # v head: rs-scaled evict, DMA-transpose to token-major
                        h = m - 2 * NH
                        vtr = qp.tile([P, TCH], f16, name="vtr", tag="vtr",
                                      bufs=2)
                        nc.vector.tensor_tensor(vtr, pss[u], rsb1[cc],
                                                mybir.AluOpType.mult)
                        for j in range(TCH // P):
                            g = half * (T // P) + cc * (TCH // P) + j
                            nc.sync.dma_start(
                                v_sb[h][:, g * P:(g + 1) * P],
                                vtr[:, j * P:(j + 1) * P], transpose=True)

        # ========== attention ([k,q] layout) + token-major Wo + RS ==========
        def attn_chunk(b, qg):
            boff = b * T
            nkt = (qg + 1) * QGRP
            esb = [[ap_.tile([P, TCH], f16, name=f"e{h}_{kt}", tag="e",
                             bufs=QT + 2)
                    for kt in range(nkt)] for h in range(NH)]
            # scores + exp (h0 then h1 so exp overlaps next head's MMs)
            for h in range(NH):
                for kt in range(nkt):
                    jd = kt - qg * QGRP  # >=0: diagonal-group k-tile
                    q0 = max(0, jd) * P
                    sc = psx.tile([P, TCH], f32, name="sc", tag="sc", bufs=2)
                    nc.tensor.matmul(
                        sc[:, q0:TCH],
                        qk_f[NH + h][:, boff + kt * P: boff + (kt + 1) * P],
                        qk_f[h][:, boff + qg * TCH + q0: boff + (qg + 1) * TCH],
                        start=True, stop=True)
                    if jd >= 0:
                        nc.vector.tensor_add(sc[:, q0:q0 + P],
                                             sc[:, q0:q0 + P], mask_sb)
                    if q0 > 0:
                        nc.vector.memset(esb[h][kt][:, 0:q0], 0.0)
                    nc.scalar.activation(
                        esb[h][kt][:, q0:TCH], sc[:, q0:TCH],
                        mybir.ActivationFunctionType.Exp,
                        bias=ebias[:, :], scale=1.0)
            # per-head: rowsum (all-ones broadcast), 1/sum on DVE overlapping
            # the AV matmuls, then normalize the output
            ot = []
            for h in range(NH):
                rbp = psx.tile([P, TCH], f32, name="rb", tag="rw", bufs=3)
                for kt in range(nkt):
                    nc.tensor.matmul(rbp, ones_pp, esb[h][kt],
                                     start=(kt == 0), stop=(kt == nkt - 1))
                r16 = ap_.tile([P, TCH], f16, name="r16", tag="r16", bufs=2)
                with nc.allow_low_precision(reason="softmax 1/rowsum"):
                    nc.vector.reciprocal(r16, rbp)
                op_ = psx.tile([P, TCH], f32, name="op", tag="acc", bufs=3)
                for kt in range(nkt):
                    nc.tensor.matmul(
                        op_, v_sb[h][:, (b * QT + kt) * P:(b * QT + kt + 1) * P],
                        esb[h][kt], start=(kt == 0), stop=(kt == nkt - 1))
                oh = ap_.tile([P, TCH], f16, name="oh", tag="oh", bufs=2)
                nc.vector.tensor_mul(oh, op_, r16)
                ot.append(oh)
            # token-major Wo partial for this 512-token chunk
            gc = b * CC + qg
            g = gc // GPA
            coff = (gc % GPA) * TCH
            for tt in range(QGRP):
                for fc in range(FC):
                    wop = psx.tile([P, TCH], f32, name="wop", tag="rw", bufs=3)
                    for h in range(NH):
                        nc.tensor.matmul(
                            wop, ot[h][:, tt * P:(tt + 1) * P],
                            wo_sb[:, h, fc * TCH:(fc + 1) * TCH],
                            start=(h == 0), stop=(h == NH - 1))
                    pt = ap_.tile([P, TCH], f16, name="pt", tag="pt", bufs=3)
                    if (tt + fc) % 2 == 0:
                        nc.scalar.copy(pt, wop)
                    else:
                        nc.vector.tensor_copy(pt, wop)
                    nc.sync.dma_start(
                        p1g[g][coff + tt * P: coff + (tt + 1) * P,
                               fc * TCH:(fc + 1) * TCH], pt)
            if (gc + 1) % GPA == 0:
                nc.gpsimd.collective_compute(
                    "ReduceScatter", mybir.AluOpType.add,
                    replica_groups=rgroups,
                    ins=[p1g[g][:, :]], outs=[a1g[g][:, :]])

        # interleaved emission: half-0 QKV, then half-1 QKV m-tiles woven
        # between batch-0 attention chunks (each fills the other's stalls),
        # then batch-1 attention
        x0, r0 = qkv_setup(0)
        for m in range(NM):
            qkv_mtile(0, m, x0, r0)
        x1s, r1s = qkv_setup(1)
        for m in range(CC):
            qkv_mtile(1, m, x1s, r1s)
            attn_chunk(0, m)
        for m in range(CC, NM):
            qkv_mtile(1, m, x1s, r1s)
        qp.release()

        mscr = tc.alloc_tile_pool(name="mscr", bufs=1)

        def mlp_prep(g):
            """x1 = x + attn for own tokens of RS group g, rmsnorm, and
            DMA-transpose into the feature-major x1nT tiles."""
            ag = mscr.tile([P, D], f16, name="ag", tag="ag", bufs=2)
            nc.sync.dma_start(ag, a1g[g][:, :])
            xg = mscr.tile([P, D], f16, name="xg", tag="xg", bufs=2)
            nc.sync.dma_start(xg, xown[g * P:(g + 1) * P, :])
            nc.vector.tensor_add(x1g[g], xg, ag)
            sqv = mscr.tile([P, D], f16, name="sqv", tag="sqv", bufs=2)
            ssq = mscr.tile([P, 1], f32, name="ssq", tag="ssq", bufs=2)
            nc.scalar.activation(sqv, x1g[g],
                                 mybir.ActivationFunctionType.Square,
                                 accum_out=ssq)
            srt = mscr.tile([P, 1], f32, name="srt", tag="srt", bufs=2)
            nc.scalar.activation(srt, ssq, mybir.ActivationFunctionType.Sqrt,
                                 bias=epsP[:, :], scale=1.0 / D)
            rsg = mscr.tile([P, 1], f32, name="rsg", tag="rsg", bufs=2)
            nc.vector.reciprocal(rsg, srt)
            x1n = mscr.tile([P, D], f16, name="x1n", tag="x1n", bufs=2)
            nc.vector.tensor_scalar_mul(x1n, x1g[g], rsg)
            for i in range(KD):
                nc.sync.dma_start(x1nT[i][:, g * P:(g + 1) * P],
                                  x1n[:, i * P:(i + 1) * P], transpose=True)

        mlp_prep(0)
        mlp_prep(1)
        attn_chunk(1, 0)
        attn_chunk(1, 1)
        mlp_prep(2)
        attn_chunk(1, 2)
        attn_chunk(1, 3)
        mlp_prep(3)
        mscr.release()
        ap_.release()
        psx.release()
        persist.release()

        # ========== local MLP on own 4x128 tokens (full FF, streamed) =======
        mp = tc.alloc_tile_pool(name="mlp", bufs=1)
        psm = tc.alloc_tile_pool(name="psumm", bufs=1, space="PSUM")
        # gate/up
        acs = []
        for m in range(FFT):
            wgt = mp.tile([P, KD * P], f16, name="wgt", tag="wgt", bufs=3)
            nc.sync.dma_start(wgt, wgd[m * P:(m + 1) * P, :])
            wut = mp.tile([P, KD * P], f16, name="wut", tag="wut", bufs=3)
            nc.sync.dma_start(wut, wud[m * P:(m + 1) * P, :])
            gp = psm.tile([P, NAR * P], f32, name="gp", tag="gp", bufs=2)
            for i in range(KD):
                nc.tensor.matmul(gp, wgt[:, i * P:(i + 1) * P], x1nT[i],
                                 start=(i == 0), stop=(i == KD - 1))
            up = psm.tile([P, NAR * P], f32, name="upp", tag="upp", bufs=2)
            for i in range(KD):
                nc.tensor.matmul(up, wut[:, i * P:(i + 1) * P], x1nT[i],
                                 start=(i == 0), stop=(i == KD - 1))
            gs = mp.tile([P, NAR * P], f16, name="gs", tag="gs", bufs=2)
            nc.scalar.activation(gs, gp, mybir.ActivationFunctionType.Silu)
            ac = mp.tile([P, NAR * P], f16, name="ac", tag="ac", bufs=FFT + 1)
            nc.vector.tensor_mul(ac, gs, up)
            acs.append(ac)
        # down + final residual
        for fc in range(FC):
            dps = [psm.tile([P, TCH], f32, name=f"dp{tt}", tag=f"dp{tt}",
                            bufs=1) for tt in range(NAR)]
            for m in range(FFT):
                wdt = mp.tile([P, TCH], f16, name="wdt", tag="wdt", bufs=4)
                nc.sync.dma_start(wdt, wdd[m * P:(m + 1) * P,
                                           fc * TCH:(fc + 1) * TCH])
                for tt in range(NAR):
                    nc.tensor.matmul(dps[tt], acs[m][:, tt * P:(tt + 1) * P],
                                     wdt, start=(m == 0), stop=(m == FFT - 1))
            for tt in range(NAR):
                yt = mp.tile([P, TCH], f32, name="yt", tag="yt", bufs=3)
                nc.vector.tensor_add(yt, dps[tt],
                                     x1g[tt][:, fc * TCH:(fc + 1) * TCH])
                nc.sync.dma_start(
                    yOut[tt * P:(tt + 1) * P, fc * TCH:(fc + 1) * TCH], yt)
        psm.release()
        mp.release()
        mkeep.release()

    nc.compile()
    return nc


# ---------------- host side ----------------

_BUILT = {}


def _get_program(cfg_key, cfg):
    if cfg_key not in _BUILT:
        _BUILT[cfg_key] = build_decoder(cfg)
    return _BUILT[cfg_key]


def _host_prep(cfg, x, position_ids, Wq, Wk, Wv, Wo, Wg, Wu, Wd, g1, g2):
    c = _derive(cfg)
    D, N, DH, HD = c["D"], c["N"], c["DH"], c["HD"]
    KD, FFT, NAR, GT = c["KD"], c["FFT"], c["NAR"], c["GT"]
    FF = cfg["FF"]
    xN = np.asarray(x).reshape(N, D).astype(np.float32)
    xN16 = xN.astype(np.float16)
    xT16 = np.ascontiguousarray(xN16.T)

    pos = np.asarray(position_ids).reshape(-1).astype(np.float32)
    inv_freq = (1.0 / (BASE ** (np.arange(0, HD, 2, dtype=np.float32) / HD)))
    ang = pos[:, None] * inv_freq[None, :]           # [N, HD/2]
    cos_f = np.concatenate([np.cos(ang), np.cos(ang)], axis=1)  # [N, HD]
    sin_f = np.concatenate([np.sin(ang), np.sin(ang)], axis=1)
    s = 1.0 / math.sqrt(HD)
    # rotate-half sign is folded into the sin tables: the kernel builds
    # rot(q)[d] = q[(d+64)%128] via two partition-shift DMAs, and the sign
    # (-1 for d < 64) lives here
    sgn = np.ones((HD, 1), np.float32)
    sgn[: HD // 2] = -1.0
    cqt = np.ascontiguousarray(cos_f.T * s).astype(np.float16)
    sqt = np.ascontiguousarray(sin_f.T * s * sgn).astype(np.float16)
    ckt = np.ascontiguousarray(cos_f.T).astype(np.float16)
    skt = np.ascontiguousarray(sin_f.T * sgn).astype(np.float16)
    # rotate-half as a permutation matrix: rot(q)[d] = sign(d) * q[(d+64) % 128]
    # lhsT layout for the PE: rotm[k, d] = sign(d) * (k == (d+64) % 128)
    rotm = np.zeros((P, P), np.float16)
    for dd in range(P):
        sgn = -1.0 if dd < P // 2 else 1.0
        rotm[(dd + P // 2) % P, dd] = sgn

    # [k, q] score layout: invalid where k > q
    ii, jj = np.indices((P, P))
    maskv = np.where(ii > jj, np.float32(-10000.0), np.float32(0.0))

    g1f = np.asarray(g1, np.float32)[:, None]
    g2f = np.asarray(g2, np.float32)[:, None]
    wqs = (g1f * np.asarray(Wq, np.float32)).astype(np.float16)
    wks = (g1f * np.asarray(Wk, np.float32)).astype(np.float16)
    wvs = (g1f * np.asarray(Wv, np.float32)).astype(np.float16)
    wgs = (g2f * np.asarray(Wg, np.float32)).astype(np.float16)
    wus = (g2f * np.asarray(Wu, np.float32)).astype(np.float16)
    wds = np.asarray(Wd, np.float32).astype(np.float16)
    wos = np.asarray(Wo, np.float32).astype(np.float16)

    # swizzle [D, M]-shaped weights so each 128-wide output tile's [P, KD*P]
    # lhsT block is one contiguous DMA: w_swz[m*P+p, k*P+j] = w[k*P+p, m*P+j]
    def _swz(w):
        mt = w.shape[1] // P
        return np.ascontiguousarray(
            w.reshape(KD, P, mt, P).transpose(2, 1, 0, 3)
            .reshape(mt * P, KD * P))

    wgd = _swz(wgs)
    wud = _swz(wus)

    in_maps = []
    for i in range(NCORES):
        qs = slice(i * DH, (i + 1) * DH)
        xo = np.concatenate(
            [xN16[g * GT + i * P: g * GT + (i + 1) * P] for g in range(NAR)],
            axis=0)
        in_maps.append({
            "xT16": xT16, "xown": np.ascontiguousarray(xo),
            "cq": cqt, "sq": sqt, "ck": ckt, "sk": skt,
            "maskd": maskv, "rotmd": rotm,
            "wqkv": _swz(
                np.concatenate([wqs[:, qs], wks[:, qs], wvs[:, qs]], axis=1)),
            "wo": np.ascontiguousarray(wos[qs, :]),
            "wgd": wgd, "wud": wud, "wdd": wds,
        })
    return in_maps


def run(cfg, inputs, **run_kwargs):
    key = tuple(sorted(cfg.items()))
    nc = _get_program(key, cfg)
    c = _derive(cfg)
    in_maps = _host_prep(cfg, **inputs)
    res = bass_utils.run_bass_kernel_spmd(
        nc, in_maps, core_ids=list(range(NCORES)), **run_kwargs)
    N, D, NAR, GT = c["N"], c["D"], c["NAR"], c["GT"]
    y = np.empty((N, D), np.float32)
    for i in range(NCORES):
        yo = np.asarray(res.results[i]["yOut"])
        for g in range(NAR):
            y[g * GT + i * P: g * GT + (i + 1) * P] = yo[g * P:(g + 1) * P]
    return y.reshape(cfg["B"], cfg["T"], cfg["D"]), res


def kernel(**inputs):
    y, _ = run(FULL_CFG, inputs)
    return y
